# revision 1
# baseline (speedup 1.0000x reference)
"""Trainium2 Bass kernel for nn_ClusterLoss (segment_reduce family).

Reference computes:
    loss = w0*omega_mean + (w1*omega_between + w2*omega_within) / bs
with (w0, w1, w2) = (1.0, 0.5, 0.5).

Algebra: with S_c the per-group column sums, t the total column sum vector,
B = sum_c ||S_c||^2 / max(m_c, 1):
    omega_within  = omega_mean - B
    omega_between = B - ||t||^2 / n
Since w1 == w2, B cancels exactly:
    loss = omega_mean + 0.5*(omega_mean - ||t||^2/n)/bs
so only one streaming pass over W is needed: sum of squares + row sums.
group_ids does not influence the result.

Device plan (per core, column shard [1024, 6250] f32, 8 chunks of 128 rows):
  - chunk-resident SBUF tiles [128 x 6250], filled by 1250-col DMAs
    (descending DMA widths on the last chunk so the stream tail is fine)
  - VectorE (DVE): reduce_sum along free axis -> per-partition row sums
  - ScalarE (ACT): Square activation with accum_out -> per-partition sq sums
  Engine op splits are aligned with DMA arrivals on the last chunk so the
  serial backlog after the final byte is minimal.
  - stats for chunks 0..6 are DMA'd out right behind the last w-DMAs (the
    DMA resource is exclusive, so emitting them mid-stream would displace
    the stream); only the last chunk's small stats DMA sits on the tail.
Host reduces the tiny [128, NSLOT] per-core stats in float64.
"""

import numpy as np

D = 1024
N_CLASSES = 50000
N_CORES = 8
P = 128
COLS = N_CLASSES // N_CORES      # 6250 columns per core
N_CHUNKS = D // P                # 8 partition chunks

BULK_DMA = (1250, 1250, 1250, 1250, 1250)
TAIL_DMA = (1250, 1250, 1250, 625, 625, 625, 313, 312)
BULK_ROW_OPS = (2500, 2500, 1250)
BULK_SQ_OPS = (2500, 2500, 1250)
TAIL_ROW_OPS = TAIL_DMA                       # DVE rowsums, aligned to DMAs
TAIL_SQ_ACT = (1250, 1250, 1250, 1250, 1250)  # ACT squares: fewer, coarser ops
# (ACT pays ~0.37us fixed cost per accum op, so five 1250-wide ops beat
# DMA-aligned ops in the tail; sq op k is ready once its two 625 DMAs land)
# NOTE: offloading tail squares to DVE via tensor_tensor_reduce looked good in
# the cost model but crashes real TRN2 (NRT_EXEC_UNIT_UNRECOVERABLE) in this
# toolchain even in its canonical broadcast-out form, so all squares stay on ACT.


def _widths_to_ranges(widths, off=0):
    out = []
    for w in widths:
        out.append((off, w))
        off += w
    return out


def _slots():
    """stats-column layout:
      section A (bulk, chunks 0..6): rows then sqs, BULK_OPS splits
      section B (chunk 7): rows (TAIL_ROW_OPS) | sqs (TAIL_SQ_ACT)
    Returns slots: slots[i] = (kind, chunk, off, width)."""
    slots = []
    for kind, ops in (("row", BULK_ROW_OPS), ("sq", BULK_SQ_OPS)):
        for c in range(N_CHUNKS - 1):
            for off, w in _widths_to_ranges(ops):
                slots.append((kind, c, off, w))
    bulk_len = len(slots)
    c = N_CHUNKS - 1
    for off, w in _widths_to_ranges(TAIL_ROW_OPS):
        slots.append(("row", c, off, w))
    for off, w in _widths_to_ranges(TAIL_SQ_ACT):
        slots.append(("sq", c, off, w))
    return slots, bulk_len


SLOTS, BULK_LEN = _slots()
NSLOT = len(SLOTS)
_SLOT_COL = {(k, c, off): i for i, (k, c, off, _w) in enumerate(SLOTS)}

LAST_RESULTS = None              # BassKernelResults of the most recent run
_NC_CACHE = {}


def _build_bass(bufs=3):
    import concourse.mybir as mybir
    from concourse import bacc
    from concourse.tile import TileContext

    nc = bacc.Bacc(
        "TRN2", target_bir_lowering=False, debug=False, num_devices=N_CORES
    )
    w = nc.declare_dram_parameter("w", [D, COLS], mybir.dt.float32, isOutput=False)
    out = nc.declare_dram_parameter(
        "stats", [P, NSLOT], mybir.dt.float32, isOutput=True
    )

    f32 = mybir.dt.float32
    last_c = N_CHUNKS - 1
    with TileContext(nc) as tc:
        with (
            tc.tile_pool(name="wpool", bufs=bufs) as wpool,
            tc.tile_pool(name="spool", bufs=1) as spool,
            tc.tile_pool(name="scratch", bufs=1) as scpool,
        ):
            stats = spool.tile([P, NSLOT], f32)
            scratch = scpool.tile([P, max(max(BULK_SQ_OPS), max(TAIL_SQ_ACT))], f32)
            for c in range(N_CHUNKS):
                last = c == last_c
                ctile = wpool.tile([P, COLS], f32, tag="wtile")
                for off, f in _widths_to_ranges(TAIL_DMA if last else BULK_DMA):
                    nc.sync.dma_start(
                        out=ctile[:, off:off + f],
                        in_=w[c * P:(c + 1) * P, off:off + f],
                    )
                if last:
                    # bulk stats drain in the free DMA slot behind the last
                    # w-DMAs (deps: chunks 0..6 compute, long since done).
                    # Emitting this any earlier displaces the w-stream on the
                    # exclusive DMA resource by its transfer time.
                    nc.sync.dma_start(
                        out=out[:, :BULK_LEN], in_=stats[:, :BULK_LEN]
                    )
                for off, f in _widths_to_ranges(
                    TAIL_ROW_OPS if last else BULK_ROW_OPS
                ):
                    sr = _SLOT_COL[("row", c, off)]
                    nc.vector.reduce_sum(
                        stats[:, sr:sr + 1],
                        ctile[:, off:off + f],
                        axis=mybir.AxisListType.X,
                    )
                for off, f in _widths_to_ranges(
                    TAIL_SQ_ACT if last else BULK_SQ_OPS
                ):
                    sq = _SLOT_COL[("sq", c, off)]
                    nc.scalar.activation(
                        scratch[:, :f],
                        ctile[:, off:off + f],
                        mybir.ActivationFunctionType.Square,
                        accum_out=stats[:, sq:sq + 1],
                    )
            # both engines retire their last op at ~the same time, so one
            # small SP DMA for the final-chunk stats beats per-engine DMAs
            # (each extra DMA pays its own completion latency)
            nc.sync.dma_start(
                out=out[:, BULK_LEN:], in_=stats[:, BULK_LEN:]
            )
    nc.compile()
    return nc


def kernel(softmax_weight, group_ids=None, batch_size=32, **_ignored):
    global LAST_RESULTS
    from concourse.bass_utils import run_bass_kernel_spmd

    W = np.ascontiguousarray(np.asarray(softmax_weight, dtype=np.float32))
    assert W.shape == (D, N_CLASSES), W.shape
    bs = float(np.asarray(batch_size))

    if "nc" not in _NC_CACHE:
        _NC_CACHE["nc"] = _build_bass()
    nc = _NC_CACHE["nc"]

    in_maps = [
        {"w": np.ascontiguousarray(W[:, k * COLS:(k + 1) * COLS])}
        for k in range(N_CORES)
    ]
    LAST_RESULTS = run_bass_kernel_spmd(nc, in_maps, core_ids=list(range(N_CORES)))

    om = 0.0
    t = np.zeros(D, np.float64)
    for r in LAST_RESULTS.results:
        st = r["stats"].astype(np.float64)          # [P, NSLOT]
        for i, (kind, c, _off, _w) in enumerate(SLOTS):
            if kind == "row":
                t[c * P:(c + 1) * P] += st[:, i]
            else:
                om += st[:, i].sum()

    T = (t @ t) / N_CLASSES
    loss = om + 0.5 * (om - T) / bs
    return np.asarray(loss, dtype=np.float32)



# revision 2
# speedup vs baseline: 1.7488x; 1.7488x over previous
"""Trainium2 Bass kernel for nn_ClusterLoss (segment_reduce family).

Reference computes:
    loss = w0*omega_mean + (w1*omega_between + w2*omega_within) / bs
with (w0, w1, w2) = (1.0, 0.5, 0.5).

Algebra: with S_c the per-group column sums, t the total column sum vector,
B = sum_c ||S_c||^2 / max(m_c, 1):
    omega_within  = omega_mean - B
    omega_between = B - ||t||^2 / n
Since w1 == w2, B cancels exactly:
    loss = omega_mean + 0.5*(omega_mean - ||t||^2/n)/bs
so only one streaming pass over W is needed: sum of squares + row sums.
group_ids does not influence the result.

Bandwidth plan: the host casts W to float16 (round-to-nearest; ~1e-4
relative noise on omega_mean, vs 2e-2 tolerance), halving HBM traffic per
core to 12.8 MB -> ~35.6us of DMA at the 360 B/ns per-core stream rate.

Compute plan per core (column shard [1024, 6250] f16, 8 chunks of 128 rows):
  - PE computes ALL of omega_mean via a Gram trick: for each 125-wide
    column slice s of each chunk, matmul(lhsT=s, rhs=s) accumulates
    s.T@s into one [125,125] PSUM bank across all 400 slices. The
    accumulated diagonal[m] is the squared-sum of every column with
    (col mod 125) == m, so trace == sum(W^2). 52 ns/slice in the cost
    model (PE is otherwise idle).
  - Row sums t (only needed for the tiny ||t||^2/n term) are split:
    ACT does cols [0:3456] per chunk via Copy activation + accum_out,
    DVE does cols [3456:6250] via reduce_sum. Both fit within the
    chunk's DMA time, so the stream stays saturated.
  - Tail chunk uses descending DMA widths and finer op splits so the
    serial backlog after the final byte is minimal; PSUM is copied to
    SBUF by DVE and drained in ONE final stats DMA.
Host reduces the tiny [128, NSLOT+125] per-core stats in float64.
"""

import numpy as np

D = 1024
N_CLASSES = 50000
N_CORES = 8
P = 128
COLS = N_CLASSES // N_CORES      # 6250 columns per core
N_CHUNKS = D // P                # 8 partition chunks

MM_W = 125                       # PE Gram slice width; 6250 = 50 * 125
MM_PER_CHUNK = COLS // MM_W

ACT_BULK = 3456                  # ACT t-share in bulk chunks; DVE takes rest
BULK_DMA = (1250, 1250, 1250, 1250, 1250)
TAIL_DMA = (1250, 1250, 1250, 1250, 625, 369, 256)
ACT_TAIL_OPS = (1250, 1250)                  # cols [0:2500)
DVE_TAIL_OPS = (1250, 1250, 625, 369, 256)   # cols [2500:6250)


def _widths_to_ranges(widths, off=0):
    out = []
    for w in widths:
        out.append((off, w))
        off += w
    return out


def _slots():
    """stats t-columns: slots[i] = (kind, chunk, off, width)."""
    slots = []
    for c in range(N_CHUNKS - 1):
        slots.append(("act", c, 0, ACT_BULK))
        slots.append(("dve", c, ACT_BULK, COLS - ACT_BULK))
    c = N_CHUNKS - 1
    for off, w in _widths_to_ranges(ACT_TAIL_OPS):
        slots.append(("act", c, off, w))
    for off, w in _widths_to_ranges(DVE_TAIL_OPS, off=sum(ACT_TAIL_OPS)):
        slots.append(("dve", c, off, w))
    return slots


SLOTS = _slots()
NSLOT = len(SLOTS)
_SLOT_COL = {(k, c, off): i for i, (k, c, off, _w) in enumerate(SLOTS)}
STATS_W = NSLOT + MM_W           # t-slots followed by the Gram diagonal block

LAST_RESULTS = None              # BassKernelResults of the most recent run
_NC_CACHE = {}


def _build_bass(bufs=3):
    import concourse.mybir as mybir
    from concourse import bacc
    from concourse.tile import TileContext

    nc = bacc.Bacc(
        "TRN2", target_bir_lowering=False, debug=False, num_devices=N_CORES
    )
    f16 = mybir.dt.float16
    f32 = mybir.dt.float32
    w = nc.declare_dram_parameter("w", [D, COLS], f16, isOutput=False)
    out = nc.declare_dram_parameter("stats", [P, STATS_W], f32, isOutput=True)

    last_c = N_CHUNKS - 1
    n_mm = N_CHUNKS * MM_PER_CHUNK
    with TileContext(nc) as tc:
        with (
            tc.tile_pool(name="wpool", bufs=bufs) as wpool,
            tc.tile_pool(name="spool", bufs=1) as spool,
            tc.tile_pool(name="scratch", bufs=1) as scpool,
            tc.psum_pool(name="pp", bufs=1) as pp,
        ):
            stats = spool.tile([P, STATS_W], f32, name="stats")
            scr = scpool.tile([P, ACT_BULK], f16, name="scr")
            ps = pp.tile([MM_W, MM_W], f32, name="ps")
            mm_i = 0
            for c in range(N_CHUNKS):
                last = c == last_c
                ctile = wpool.tile([P, COLS], f16, tag="wtile")
                for off, f in _widths_to_ranges(TAIL_DMA if last else BULK_DMA):
                    nc.sync.dma_start(
                        out=ctile[:, off:off + f],
                        in_=w[c * P:(c + 1) * P, off:off + f],
                    )
                # PE: Gram accumulation (diagonal -> sum of squares)
                for s in range(MM_PER_CHUNK):
                    sl = ctile[:, s * MM_W:(s + 1) * MM_W]
                    nc.tensor.matmul(
                        ps[:, :], sl, sl,
                        start=(mm_i == 0), stop=(mm_i == n_mm - 1),
                    )
                    mm_i += 1
                # t row sums: ACT share then DVE share
                if last:
                    act_ops = _widths_to_ranges(ACT_TAIL_OPS)
                    dve_ops = _widths_to_ranges(DVE_TAIL_OPS, off=sum(ACT_TAIL_OPS))
                else:
                    act_ops = [(0, ACT_BULK)]
                    dve_ops = [(ACT_BULK, COLS - ACT_BULK)]
                for off, f in act_ops:
                    sa = _SLOT_COL[("act", c, off)]
                    nc.scalar.activation(
                        scr[:, :f],
                        ctile[:, off:off + f],
                        mybir.ActivationFunctionType.Copy,
                        accum_out=stats[:, sa:sa + 1],
                    )
                for off, f in dve_ops:
                    sd = _SLOT_COL[("dve", c, off)]
                    nc.vector.reduce_sum(
                        stats[:, sd:sd + 1],
                        ctile[:, off:off + f],
                        axis=mybir.AxisListType.X,
                    )
            # PSUM diag block -> SBUF stats, then one DMA for everything
            nc.vector.tensor_copy(stats[0:MM_W, NSLOT:STATS_W], ps[:, :])
            nc.sync.dma_start(out=out[:, :], in_=stats[:, :])
    nc.compile()
    return nc


def kernel(softmax_weight, group_ids=None, batch_size=32, **_ignored):
    global LAST_RESULTS
    from concourse.bass_utils import run_bass_kernel_spmd

    W = np.asarray(softmax_weight)
    assert W.shape == (D, N_CLASSES), W.shape
    bs = float(np.asarray(batch_size))
    W16 = W.astype(np.float16)

    if "nc" not in _NC_CACHE:
        _NC_CACHE["nc"] = _build_bass()
    nc = _NC_CACHE["nc"]

    in_maps = [
        {"w": np.ascontiguousarray(W16[:, k * COLS:(k + 1) * COLS])}
        for k in range(N_CORES)
    ]
    LAST_RESULTS = run_bass_kernel_spmd(nc, in_maps, core_ids=list(range(N_CORES)))

    om = 0.0
    t = np.zeros(D, np.float64)
    for r in LAST_RESULTS.results:
        st = r["stats"].astype(np.float64)          # [P, STATS_W]
        om += np.trace(st[0:MM_W, NSLOT:STATS_W])
        for i, (_kind, c, _off, _w) in enumerate(SLOTS):
            t[c * P:(c + 1) * P] += st[:, i]

    T = (t @ t) / N_CLASSES
    loss = om + 0.5 * (om - T) / bs
    return np.asarray(loss, dtype=np.float32)


# revision 3
# speedup vs baseline: 2.0265x; 1.1588x over previous
"""Trainium2 Bass kernel for nn_ClusterLoss (segment_reduce family).

Reference computes:
    loss = w0*omega_mean + (w1*omega_between + w2*omega_within) / bs
with (w0, w1, w2) = (1.0, 0.5, 0.5).

Algebra: with S_c the per-group column sums, t the total column sum vector,
B = sum_c ||S_c||^2 / max(m_c, 1):
    omega_within  = omega_mean - B
    omega_between = B - ||t||^2 / n
Since w1 == w2, B cancels exactly:
    loss = omega_mean + 0.5*(omega_mean - ||t||^2/n)/bs
so only one streaming pass over W is needed: sum of squares + row sums.
group_ids does not influence the result.

Bandwidth plan: the host casts columns [0:F16) of each shard to float16 and
columns [F16:COLS) to fp8 e4m3 (prescaled by 128; TRN e4m3 max-normal 240).
Quantization is round-to-nearest (unbiased); the omega_mean error is ~1e-4
relative vs the 2e-2 tolerance. Per-core HBM traffic drops from 25.6 MB (f32)
to 10.6 MB -> ~30.2us of DMA at the 360 B/ns per-core stream rate.

Compute plan per core (shard [1024, 6250], 8 chunks of 128 rows):
  - PE computes ALL of omega_mean via a Gram trick: for each 125-wide
    column slice s, matmul(lhsT=s, rhs=s) accumulates s.T@s into a PSUM
    bank (one bank per dtype since fp8 is prescaled) across all slices.
    The accumulated diagonal[m] is the squared-sum of every column with
    (col mod 125) == m, so trace == sum(W^2). 52 ns/slice in the cost
    model; PE is otherwise idle. fp8*fp8 products are exact in f32, so
    the only fp8 error is input quantization.
  - Row sums t (only needed for the tiny ||t||^2/n term) are split per
    chunk: ACT does the fp8 tile + one f16 slice via Copy activation +
    accum_out, DVE reduces the remaining f16 slices. Both roughly fit
    within the chunk's DMA time, so the stream stays saturated.
  - Tail chunk reorders DMAs (fp8 + late f16 slices first) and splits
    ops finer so the serial backlog after the final byte is minimal.
    PE's fp8 Gram group closes early in the tail chunk; DVE copies its
    PSUM mid-chunk, ACT copies the f16 Gram at the end; ONE final DMA
    drains stats + both diagonals.
Host reduces the tiny [128, STATS_W] per-core stats in float64, rescaling
the fp8 contributions (t slots by 1/128, Gram diag by 1/16384).
"""

import numpy as np

D = 1024
N_CLASSES = 50000
N_CORES = 8
P = 128
COLS = N_CLASSES // N_CORES      # 6250 columns per core
N_CHUNKS = D // P                # 8 partition chunks

MM_W = 125                       # PE Gram slice width
F16 = 4375                       # f16 columns per core (35 * 125)
F8 = COLS - F16                  # fp8 columns per core (15 * 125) = 1875
F8_SCALE = 128.0                 # host premultiplies fp8 values by this

# bulk chunks: DMA order f8 | f16 s1..s4; ACT <- f8 + s1, DVE <- s2 s3 s4
BULK_F16_DMA = (1250, 1250, 1250, 625)
# tail chunk: f8 first, then f16 with the last range split fine:
# s4[3750:4375] early, then s1, s2, then s3 split (625, 313, 312)
TAIL_F16_DMA = ((3750, 625), (0, 1250), (1250, 1250), (2500, 625),
                (3125, 313), (3438, 312))

LAST_RESULTS = None              # BassKernelResults of the most recent run
_NC_CACHE = {}

# t-slot bookkeeping: list of (engine, chunk, kind, off, width); kind "f16"/"f8"
def _tail_act_dve():
    act = [("f8", 0, F8), ("f16", 0, 1250)]
    dve = [("f16", 3750, 625), ("f16", 1250, 1250), ("f16", 2500, 625),
           ("f16", 3125, 313), ("f16", 3438, 312)]
    return act, dve


def _slots():
    slots = []
    for c in range(N_CHUNKS - 1):
        slots.append(("act", c, "f8", 0, F8))
        slots.append(("act", c, "f16", 0, 1250))
        slots.append(("dve", c, "f16", 1250, 1250))
        slots.append(("dve", c, "f16", 2500, 1250))
        slots.append(("dve", c, "f16", 3750, 625))
    c = N_CHUNKS - 1
    act, dve = _tail_act_dve()
    for kind, off, w in act:
        slots.append(("act", c, kind, off, w))
    for kind, off, w in dve:
        slots.append(("dve", c, kind, off, w))
    return slots


SLOTS = _slots()
NSLOT = len(SLOTS)
_SLOT_COL = {(e, c, k, off): i for i, (e, c, k, off, _w) in enumerate(SLOTS)}
STATS_W = NSLOT + 2 * MM_W       # t-slots | f16 Gram diag | f8 Gram diag

def _build_bass(bufs=3):
    import concourse.mybir as mybir
    from concourse import bacc
    from concourse.tile import TileContext

    nc = bacc.Bacc(
        "TRN2", target_bir_lowering=False, debug=False, num_devices=N_CORES
    )
    f16 = mybir.dt.float16
    f8 = mybir.dt.float8e4
    f32 = mybir.dt.float32
    w16 = nc.declare_dram_parameter("w16", [D, F16], f16, isOutput=False)
    w8 = nc.declare_dram_parameter("w8", [D, F8], f8, isOutput=False)
    out = nc.declare_dram_parameter("stats", [P, STATS_W], f32, isOutput=True)

    last_c = N_CHUNKS - 1
    n16 = F16 // MM_W
    n8 = F8 // MM_W
    Copy = mybir.ActivationFunctionType.Copy
    X = mybir.AxisListType.X
    with TileContext(nc) as tc:
        with (
            tc.tile_pool(name="wpool", bufs=bufs) as wpool,
            tc.tile_pool(name="w8pool", bufs=bufs) as w8pool,
            tc.tile_pool(name="spool", bufs=1) as spool,
            tc.tile_pool(name="scratch", bufs=1) as scpool,
            tc.psum_pool(name="pp", bufs=2) as pp,
        ):
            stats = spool.tile([P, STATS_W], f32, name="stats")
            scr = scpool.tile([P, F8], f16, name="scr")
            ps16 = pp.tile([MM_W, MM_W], f32, name="ps16")
            ps8 = pp.tile([MM_W, MM_W], f32, name="ps8")
            for c in range(N_CHUNKS):
                last = c == last_c
                t16 = wpool.tile([P, F16], f16, tag="w16t")
                t8 = w8pool.tile([P, F8], f8, tag="w8t")
                rows = slice(c * P, (c + 1) * P)
                # DMAs: f8 tile first, then f16 slices
                nc.sync.dma_start(out=t8[:, :], in_=w8[rows, :])
                f16_ranges = (TAIL_F16_DMA if last else
                              _widths_to_ranges(BULK_F16_DMA))
                for off, f in f16_ranges:
                    nc.sync.dma_start(
                        out=t16[:, off:off + f], in_=w16[rows, off:off + f]
                    )
                # PE Gram: f8 slices first (group closes early in tail chunk)
                for s in range(n8):
                    sl = t8[:, s * MM_W:(s + 1) * MM_W]
                    nc.tensor.matmul(
                        ps8[:, :], sl, sl,
                        start=(c == 0 and s == 0),
                        stop=(last and s == n8 - 1),
                        skip_group_check=True,
                    )
                for s in range(n16):
                    sl = t16[:, s * MM_W:(s + 1) * MM_W]
                    nc.tensor.matmul(
                        ps16[:, :], sl, sl,
                        start=(c == 0 and s == 0),
                        stop=(last and s == n16 - 1),
                        skip_group_check=True,
                    )
                if last:
                    # f8 Gram done once ps8's stop-matmul retires; DVE drains
                    # it mid-chunk while the f16 stream is still landing
                    nc.vector.tensor_copy(
                        stats[0:MM_W, NSLOT + MM_W:STATS_W], ps8[:, :]
                    )
                # t row sums
                if last:
                    act_ops, dve_ops = _tail_act_dve()
                else:
                    act_ops = [("f8", 0, F8), ("f16", 0, 1250)]
                    dve_ops = [("f16", 1250, 1250), ("f16", 2500, 1250),
                               ("f16", 3750, 625)]
                for kind, off, f in act_ops:
                    sa = _SLOT_COL[("act", c, kind, off)]
                    src = t8 if kind == "f8" else t16
                    nc.scalar.activation(
                        scr[:, :f], src[:, off:off + f], Copy,
                        accum_out=stats[:, sa:sa + 1],
                    )
                for kind, off, f in dve_ops:
                    sd = _SLOT_COL[("dve", c, kind, off)]
                    src = t8 if kind == "f8" else t16
                    nc.vector.reduce_sum(
                        stats[:, sd:sd + 1], src[:, off:off + f], axis=X
                    )
            # f16 Gram diag -> SBUF (ACT, which is idle by then), one DMA
            nc.scalar.copy(stats[0:MM_W, NSLOT:NSLOT + MM_W], ps16[:, :])
            nc.sync.dma_start(out=out[:, :], in_=stats[:, :])
    nc.compile()
    return nc


def _widths_to_ranges(widths, off=0):
    out = []
    for w in widths:
        out.append((off, w))
        off += w
    return out


def kernel(softmax_weight, group_ids=None, batch_size=32, **_ignored):
    global LAST_RESULTS
    import ml_dtypes
    from concourse.bass_utils import run_bass_kernel_spmd

    W = np.asarray(softmax_weight)
    assert W.shape == (D, N_CLASSES), W.shape
    bs = float(np.asarray(batch_size))

    if "nc" not in _NC_CACHE:
        _NC_CACHE["nc"] = _build_bass()
    nc = _NC_CACHE["nc"]

    in_maps = []
    for k in range(N_CORES):
        sh = W[:, k * COLS:(k + 1) * COLS]
        in_maps.append({
            "w16": np.ascontiguousarray(sh[:, :F16]).astype(np.float16),
            "w8": (np.ascontiguousarray(sh[:, F16:]) * F8_SCALE).astype(
                ml_dtypes.float8_e4m3),
        })
    LAST_RESULTS = run_bass_kernel_spmd(nc, in_maps, core_ids=list(range(N_CORES)))

    om = 0.0
    t = np.zeros(D, np.float64)
    s8 = 1.0 / F8_SCALE
    for r in LAST_RESULTS.results:
        st = r["stats"].astype(np.float64)          # [P, STATS_W]
        om += np.trace(st[0:MM_W, NSLOT:NSLOT + MM_W])
        om += np.trace(st[0:MM_W, NSLOT + MM_W:STATS_W]) * s8 * s8
        for i, (_e, c, kind, _off, _w) in enumerate(SLOTS):
            t[c * P:(c + 1) * P] += st[:, i] * (s8 if kind == "f8" else 1.0)

    T = (t @ t) / N_CLASSES
    loss = om + 0.5 * (om - T) / bs
    return np.asarray(loss, dtype=np.float32)
